# revision 8
# baseline (speedup 1.0000x reference)
"""Dense transformer block on 8 NeuronCores.

Sharding: cores 0-3 handle batch 0, cores 4-7 batch 1. Within a batch group,
core rank r owns heads [4r, 4r+4) for attention (Megatron-style) and rows
{512j + 128r : j in 0..3} for the FFN, connected by a 4-rank ReduceScatter of
the partial attention output. LayerNorms are computed on the owned rows.

Per core the attention probabilities are computed twice on the tensor engine:
once as S[q,k] (fp32r, additive mask folded in as an identity-matmul chunk of
the PSUM accumulation, exp with accumulated row sums -> normalized tiles DMA'd
straight to the attn_prob output) and once as S^T[k,q] (exp'd, masked
multiplicatively on GPSIMD, feeding the P.V matmul as the k-on-partitions
operand, normalization deferred to the context copyback).
"""

import math

import numpy as np
import ml_dtypes

import concourse.bass as bass
import concourse.mybir as mybir
import concourse.tile as tile
from concourse.bass_utils import run_bass_kernel_spmd

BF16 = ml_dtypes.bfloat16

H, DH = 16, 64
D, PF = 1024, 4096
B, S = 2, 2048
HL = 4            # heads per core
ROWS = 512        # rows owned per core
EPS = 1e-5
NEG = -1.0e9
P = 128

f32 = mybir.dt.float32
f32r = mybir.dt.float32r
bf16 = mybir.dt.bfloat16


def _split_multi_waits(nc):
    """This walrus build accepts at most one inline sem wait per instruction;
    hoist extras onto standalone EventSemaphore instructions."""
    for f in nc.m.functions:
        for blk in f.blocks:
            new_insts = []
            for ins in blk.instructions:
                si = ins.sync_info
                if si is not None and si.on_wait is not None and len(si.on_wait) > 1:
                    waits = list(si.on_wait)
                    for j, w in enumerate(waits[:-1]):
                        new_insts.append(
                            mybir.InstEventSemaphore(
                                name=f"{ins.name}-hw{j}",
                                engine=ins.engine,
                                ins=[],
                                outs=[],
                                sync_info=mybir.SyncInfo(on_wait=[w], on_update=[]),
                            )
                        )
                    ins.sync_info = mybir.SyncInfo(
                        on_wait=[waits[-1]], on_update=list(si.on_update or [])
                    )
                new_insts.append(ins)
            blk.instructions = new_insts


def _layernorm(nc, pool, x_sb, g_sb, b_sb, scratch_tag):
    """LN over the free axis of x_sb [128, D] (fp32, SBUF). Returns a new tile."""
    mu = pool.tile([P, 1], f32, tag=f"{scratch_tag}_mu")
    nc.vector.tensor_reduce(mu[:], x_sb[:], axis=mybir.AxisListType.X,
                            op=mybir.AluOpType.add)
    negmu = pool.tile([P, 1], f32, tag=f"{scratch_tag}_negmu")
    nc.vector.tensor_scalar(negmu[:], mu[:], -1.0 / D, None,
                            op0=mybir.AluOpType.mult)
    sq = pool.tile([P, D], f32, tag=f"{scratch_tag}_sq")
    ssq = pool.tile([P, 1], f32, tag=f"{scratch_tag}_ssq")
    nc.scalar.activation(sq[:], x_sb[:], mybir.ActivationFunctionType.Square,
                         bias=negmu[:], accum_out=ssq[:])
    std = pool.tile([P, 1], f32, tag=f"{scratch_tag}_std")
    nc.vector.tensor_scalar(std[:], ssq[:], 1.0 / D, EPS,
                            op0=mybir.AluOpType.mult, op1=mybir.AluOpType.add)
    nc.scalar.sqrt(std[:], std[:])
    rstd = pool.tile([P, 1], f32, tag=f"{scratch_tag}_rstd")
    nc.vector.reciprocal(rstd[:], std[:])
    xn = pool.tile([P, D], f32, tag=f"{scratch_tag}_xn")
    nc.vector.tensor_scalar(xn[:], x_sb[:], negmu[:], rstd[:],
                            op0=mybir.AluOpType.add, op1=mybir.AluOpType.mult)
    nc.vector.tensor_tensor(xn[:], xn[:], g_sb[:], mybir.AluOpType.mult)
    nc.vector.tensor_tensor(xn[:], xn[:], b_sb[:], mybir.AluOpType.add)
    return xn


def build_nc():
    nc = bass.Bass(num_devices=8)

    xt = nc.dram_tensor("xt", [9, P, S], f32r, kind="ExternalInput")
    xrows = nc.dram_tensor("xrows", [ROWS, D], f32, kind="ExternalInput")
    wq = nc.dram_tensor("wq", [9, P, 256], f32r, kind="ExternalInput")
    wk = nc.dram_tensor("wk", [9, P, 256], f32r, kind="ExternalInput")
    wv = nc.dram_tensor("wv", [9, P, 256], f32r, kind="ExternalInput")
    wo = nc.dram_tensor("wo", [HL, DH, D], bf16, kind="ExternalInput")
    w1 = nc.dram_tensor("w1", [9, P, PF], bf16, kind="ExternalInput")
    w2 = nc.dram_tensor("w2", [33, P, D], bf16, kind="ExternalInput")
    maskq = nc.dram_tensor("maskq", [16, P, S], bf16, kind="ExternalInput")
    maskm = nc.dram_tensor("maskm", [16, P, S], bf16, kind="ExternalInput")
    identb = nc.dram_tensor("identb", [P, P], bf16, kind="ExternalInput")
    onesrow = nc.dram_tensor("onesrow", [P, ROWS], bf16, kind="ExternalInput")
    lng1 = nc.dram_tensor("lng1", [P, D], f32, kind="ExternalInput")
    lnb1 = nc.dram_tensor("lnb1", [P, D], f32, kind="ExternalInput")
    lng2 = nc.dram_tensor("lng2", [P, D], f32, kind="ExternalInput")
    lnb2 = nc.dram_tensor("lnb2", [P, D], f32, kind="ExternalInput")

    apr = nc.dram_tensor("apr", [HL, S, S], f32, kind="ExternalOutput")
    outp = nc.dram_tensor("outp", [ROWS, D], f32, kind="ExternalOutput")

    with tile.TileContext(nc) as tc:
        with (
            tc.tile_pool(name="persist", bufs=1) as pers,
            tc.tile_pool(name="dram", bufs=1, space="DRAM") as dram,
        ):
            acts_ctx = tc.tile_pool(name="acts", bufs=1)
            acts = acts_ctx.__enter__()
            ctxp_ctx = tc.tile_pool(name="ctxp", bufs=1)
            ctxp = ctxp_ctx.__enter__()
            ib = pers.tile([P, P], bf16)
            nc.sync.dma_start(ib[:], identb[:])
            ones_sb = pers.tile([P, ROWS], bf16)
            nc.sync.dma_start(ones_sb[:], onesrow[:])

            # activation storage (released when `acts`/`ctxp` close)
            qt_sb = [acts.tile([P, S], f32r, name=f"qt{i}") for i in range(2)]
            kt_sb = [acts.tile([P, S], f32r, name=f"kt{i}") for i in range(2)]
            v_sb = [acts.tile([P, 256], bf16, name=f"v{i}") for i in range(16)]
            sums = acts.tile([P, 64], f32)
            recips = acts.tile([P, 64], f32)
            ctxraw = [ctxp.tile([DH, S], f32, name=f"cr{i}") for i in range(HL)]
            ctxs = [ctxp.tile([DH, S], bf16, name=f"cs{i}") for i in range(HL)]

            # ---- Phase 1: projections (QT/KT fp32r, V bf16) ----
            with tc.tile_pool(name="p1sb", bufs=1) as p1sb:
                xt_sb = [p1sb.tile([P, S], f32r, name=f"xt{c}") for c in range(9)]
                for c in range(9):
                    nc.sync.dma_start(xt_sb[c][:], xt[c])
                wq_sb = [p1sb.tile([P, 256], f32r, name=f"wq{c}") for c in range(9)]
                wk_sb = [p1sb.tile([P, 256], f32r, name=f"wk{c}") for c in range(9)]
                wv_sb = [p1sb.tile([P, 256], f32r, name=f"wv{c}") for c in range(9)]
                for c in range(9):
                    nc.sync.dma_start(wq_sb[c][:], wq[c])
                    nc.sync.dma_start(wk_sb[c][:], wk[c])
                    nc.sync.dma_start(wv_sb[c][:], wv[c])

                with tc.tile_pool(name="p1ps", bufs=2, space="PSUM") as p1ps:
                    for w_sb, dst in ((wq_sb, qt_sb), (wk_sb, kt_sb)):
                        for mt in range(2):
                            ps = p1ps.tile([P, S], f32, tag="qkps")
                            for nk in range(4):
                                for c in range(9):
                                    nc.tensor.matmul(
                                        ps[:, nk * 512:(nk + 1) * 512],
                                        w_sb[c][:, mt * P:(mt + 1) * P],
                                        xt_sb[c][:, nk * 512:(nk + 1) * 512],
                                        start=(c == 0), stop=(c == 8),
                                    )
                            nc.vector.tensor_copy(dst[mt][:], ps[:])

                with tc.tile_pool(name="p1pv", bufs=4, space="PSUM") as p1pv:
                    for st in range(16):
                        ps = p1pv.tile([P, 256], f32, tag="vps")
                        for c in range(9):
                            nc.tensor.matmul(
                                ps[:],
                                xt_sb[c][:, st * P:(st + 1) * P],
                                wv_sb[c][:],
                                start=(c == 0), stop=(c == 8),
                            )
                        nc.vector.tensor_copy(v_sb[st][:], ps[:])

            # ---- Phase 2: [q,k] logits + softmax -> attn_prob out ----
            with (
                tc.tile_pool(name="p2sb", bufs=3) as p2sb,
                tc.tile_pool(name="p2ps", bufs=2, space="PSUM") as p2ps,
            ):
                for qt in range(16):
                    mq = p2sb.tile([P, S], bf16, tag="mq")
                    nc.sync.dma_start(mq[:], maskq[qt])
                    for h in range(HL):
                        idx = h * 16 + qt
                        pb = (h % 2) * 64
                        ps = p2ps.tile([P, S], f32, tag="lg")
                        for nk in range(4):
                            sl = slice(nk * 512, (nk + 1) * 512)
                            nc.tensor.matmul(ps[:, sl], ib[:], mq[:, sl],
                                             start=True, stop=False)
                            nc.tensor.matmul(
                                ps[:, sl],
                                qt_sb[h // 2][pb:pb + 64, qt * P:(qt + 1) * P],
                                kt_sb[h // 2][pb:pb + 64, sl],
                                start=False, stop=True,
                            )
                        e = p2sb.tile([P, S], f32, tag="e")
                        nc.scalar.activation(
                            e[:], ps[:], mybir.ActivationFunctionType.Exp,
                            accum_out=sums[:, idx:idx + 1],
                        )
                        nc.vector.reciprocal(recips[:, idx:idx + 1],
                                             sums[:, idx:idx + 1])
                        nc.vector.tensor_scalar(
                            e[:], e[:], recips[:, idx:idx + 1], None,
                            op0=mybir.AluOpType.mult,
                        )
                        nc.sync.dma_start(apr[h, qt * P:(qt + 1) * P, :], e[:])

            # ---- Phase 3: S^T / exp / mask / P.V (raw context) ----
            with (
                tc.tile_pool(name="p3sb", bufs=3) as p3sb,
                tc.tile_pool(name="p3ps", bufs=2, space="PSUM") as p3ps,
                tc.tile_pool(name="p3ps1", bufs=1, space="PSUM") as p3ps1,
            ):
                for h in range(HL):
                    pb = (h % 2) * 64
                    ctxps = p3ps1.tile([DH, S], f32, tag="ctx")
                    for kt in range(16):
                        mt = p3sb.tile([P, S], bf16, tag="mt")
                        nc.sync.dma_start(mt[:], maskm[kt])
                        for hf in range(2):
                            stps = p3ps.tile([P, 1024], f32, tag="st")
                            for c2 in range(2):
                                qsl = slice(hf * 1024 + c2 * 512,
                                            hf * 1024 + (c2 + 1) * 512)
                                nc.tensor.matmul(
                                    stps[:, c2 * 512:(c2 + 1) * 512],
                                    kt_sb[h // 2][pb:pb + 64, kt * P:(kt + 1) * P],
                                    qt_sb[h // 2][pb:pb + 64, qsl],
                                    start=True, stop=True,
                                )
                            eb = p3sb.tile([P, 1024], bf16, tag="eb")
                            nc.scalar.activation(
                                eb[:], stps[:], mybir.ActivationFunctionType.Exp)
                            em = p3sb.tile([P, 1024], bf16, tag="em")
                            nc.gpsimd.tensor_tensor(
                                em[:], eb[:],
                                mt[:, hf * 1024:(hf + 1) * 1024],
                                mybir.AluOpType.mult,
                            )
                            for c2 in range(2):
                                osl = slice(hf * 1024 + c2 * 512,
                                            hf * 1024 + (c2 + 1) * 512)
                                nc.tensor.matmul(
                                    ctxps[:, osl],
                                    v_sb[kt][:, h * DH:(h + 1) * DH],
                                    em[:, c2 * 512:(c2 + 1) * 512],
                                    start=(kt == 0), stop=(kt == 15),
                                )
                    nc.vector.tensor_copy(ctxraw[h][:], ctxps[:])

            # ---- Phase 4: normalize context, Wo, ReduceScatter ----
            rs_out_sb = [None] * 4
            with (
                tc.tile_pool(name="p4sb", bufs=3) as p4sb,
                tc.tile_pool(name="p4c", bufs=1) as p4c,
                tc.tile_pool(name="p4ps", bufs=2, space="PSUM") as p4ps,
            ):
                wo_sb = [p4c.tile([DH, D], bf16, name=f"wo{h}")
                         for h in range(HL)]
                for h in range(HL):
                    nc.sync.dma_start(wo_sb[h][:], wo[h])

                # row-replicated reciprocal tiles via column-replicate+transpose
                for h in range(HL):
                    for qt in range(16):
                        idx = h * 16 + qt
                        rep = p4sb.tile([P, P], bf16, tag="rep")
                        nc.vector.tensor_scalar(
                            rep[:], ib[:], 0.0, recips[:, idx:idx + 1],
                            op0=mybir.AluOpType.mult, op1=mybir.AluOpType.add,
                        )
                        rpt = p4ps.tile([P, P], bf16, tag="rpt")
                        nc.tensor.transpose(rpt[:], rep[:], ib[:])
                        bcq = p4sb.tile([P, P], bf16, tag="bcq")
                        nc.vector.tensor_copy(bcq[:], rpt[:])
                        nc.vector.tensor_tensor(
                            ctxs[h][:, qt * P:(qt + 1) * P],
                            ctxraw[h][:, qt * P:(qt + 1) * P],
                            bcq[0:DH, :],
                            mybir.AluOpType.mult,
                        )

                rsin = [dram.tile([ROWS, D], f32, name=f"rsin{j}")
                        for j in range(4)]
                rsout = [dram.tile([P, D], f32, name=f"rsout{j}")
                         for j in range(4)]
                for j in range(4):
                    for q4 in range(4):
                        qt = j * 4 + q4
                        aops = p4ps.tile([P, D], f32, tag="ao")
                        for nk in range(2):
                            sl = slice(nk * 512, (nk + 1) * 512)
                            for h in range(HL):
                                nc.tensor.matmul(
                                    aops[:, sl],
                                    ctxs[h][:, qt * P:(qt + 1) * P],
                                    wo_sb[h][:, sl],
                                    start=(h == 0), stop=(h == HL - 1),
                                )
                        aosb = p4sb.tile([P, D], f32, tag="aosb")
                        nc.vector.tensor_copy(aosb[:], aops[:])
                        nc.sync.dma_start(rsin[j][q4 * P:(q4 + 1) * P, :], aosb[:])
                    nc.gpsimd.collective_compute(
                        "ReduceScatter",
                        mybir.AluOpType.add,
                        replica_groups=[[0, 1, 2, 3], [4, 5, 6, 7]],
                        ins=[rsin[j].opt()],
                        outs=[rsout[j].opt()],
                    )
                    rs_out_sb[j] = pers.tile([P, D], f32, name=f"rss{j}")
                    nc.sync.dma_start(rs_out_sb[j][:], rsout[j][:])

            ctxp_ctx.__exit__(None, None, None)
            acts_ctx.__exit__(None, None, None)

            # ---- Phase 5: LN1 + FFN + LN2 ----
            with (
                tc.tile_pool(name="p5sb", bufs=2) as p5sb,
                tc.tile_pool(name="p5c", bufs=1) as p5c,
            ):
                g1 = p5c.tile([P, D], f32)
                b1 = p5c.tile([P, D], f32)
                g2 = p5c.tile([P, D], f32)
                b2 = p5c.tile([P, D], f32)
                nc.sync.dma_start(g1[:], lng1[:])
                nc.sync.dma_start(b1[:], lnb1[:])
                nc.sync.dma_start(g2[:], lng2[:])
                nc.sync.dma_start(b2[:], lnb2[:])

                x1 = [p5c.tile([P, D], f32, name=f"x1_{j}") for j in range(4)]
                x1t = [p5c.tile([P, ROWS], bf16, name=f"x1t{c}") for c in range(8)]
                p5ps_ctx = tc.tile_pool(name="p5ps", bufs=2, space="PSUM")
                p5ps = p5ps_ctx.__enter__()
                for j in range(4):
                    xr = p5sb.tile([P, D], f32, tag="xr")
                    nc.sync.dma_start(xr[:], xrows[j * P:(j + 1) * P, :])
                    nc.vector.tensor_tensor(xr[:], xr[:], rs_out_sb[j][:],
                                            mybir.AluOpType.add)
                    xn = _layernorm(nc, p5sb, xr, g1, b1, "ln1")
                    nc.vector.tensor_copy(x1[j][:], xn[:])
                    x1b = p5sb.tile([P, D], bf16, tag="x1b")
                    nc.vector.tensor_copy(x1b[:], xn[:])
                    for c in range(8):
                        tp = p5ps.tile([P, P], bf16, tag="tp")
                        nc.tensor.transpose(tp[:], x1b[:, c * P:(c + 1) * P], ib[:])
                        nc.vector.tensor_copy(x1t[c][:, j * P:(j + 1) * P], tp[:])

                p5ps_ctx.__exit__(None, None, None)

                ht = [p5c.tile([P, ROWS], bf16, name=f"ht{m}") for m in range(32)]
                with tc.tile_pool(name="p5h", bufs=1, space="PSUM") as p5h:
                    for mb in range(4):
                        hps = [p5h.tile([P, ROWS], f32, name=f"h{m}")
                               for m in range(8)]
                        for c in range(9):
                            w1s = p5sb.tile([P, 1024], bf16, tag="w1s")
                            nc.sync.dma_start(
                                w1s[:], w1[c, :, mb * 1024:(mb + 1) * 1024])
                            rhs = x1t[c] if c < 8 else ones_sb
                            for m in range(8):
                                nc.tensor.matmul(
                                    hps[m][:],
                                    w1s[:, m * P:(m + 1) * P],
                                    rhs[:],
                                    start=(c == 0), stop=(c == 8),
                                )
                        for m in range(8):
                            nc.scalar.activation(
                                ht[mb * 8 + m][:], hps[m][:],
                                mybir.ActivationFunctionType.Relu,
                            )

                with tc.tile_pool(name="p5o", bufs=1, space="PSUM") as p5o:
                    o2ps = [p5o.tile([P, D], f32, name=f"o2_{r}") for r in range(4)]
                    for c in range(33):
                        w2s = p5sb.tile([P, D], bf16, tag="w2s")
                        nc.sync.dma_start(w2s[:], w2[c])
                        for r in range(4):
                            lh = (ht[c][:, r * P:(r + 1) * P] if c < 32
                                  else ones_sb[:, r * P:(r + 1) * P])
                            for nk in range(2):
                                sl = slice(nk * 512, (nk + 1) * 512)
                                nc.tensor.matmul(
                                    o2ps[r][:, sl], lh, w2s[:, sl],
                                    start=(c == 0), stop=(c == 32),
                                )
                    for r in range(4):
                        x2 = p5sb.tile([P, D], f32, tag="x2")
                        nc.vector.tensor_copy(x2[:], o2ps[r][:])
                        nc.vector.tensor_tensor(x2[:], x2[:], x1[r][:],
                                                mybir.AluOpType.add)
                        on = _layernorm(nc, p5sb, x2, g2, b2, "ln2")
                        nc.sync.dma_start(outp[r * P:(r + 1) * P, :], on[:])

    _split_multi_waits(nc)
    return nc


_NC_CACHE = {}


def _get_nc():
    if "nc" not in _NC_CACHE:
        _NC_CACHE["nc"] = build_nc()
    return _NC_CACHE["nc"]


def kernel(inputs, padding_mask, Wq, bq, Wk, bk, Wv, bv, Wo, bo,
           W1, bf1, W2, bf2, ln1_g, ln1_b, ln2_g, ln2_b, **kw):
    inputs = np.asarray(inputs, dtype=np.float32)
    padding_mask = np.asarray(padding_mask)
    Wq, Wk, Wv = (np.asarray(a, np.float32) for a in (Wq, Wk, Wv))
    Wo, W1, W2 = (np.asarray(a, np.float32) for a in (Wo, W1, W2))
    bq, bk, bv, bo = (np.asarray(a, np.float32) for a in (bq, bk, bv, bo))
    bf1, bf2 = np.asarray(bf1, np.float32), np.asarray(bf2, np.float32)

    ident = np.eye(P, dtype=BF16)
    onesr = np.zeros((P, ROWS), dtype=BF16)
    onesr[0, :] = 1.0

    def ext9(w, b, cols):
        e = np.zeros((9 * P, cols), np.float32)
        e[:D] = w
        e[D] = b
        return e.reshape(9, P, cols)

    w1e = np.zeros((9 * P, PF), np.float32)
    w1e[:D] = W1
    w1e[D] = bf1
    w1e = w1e.astype(BF16).reshape(9, P, PF)
    w2e = np.zeros((33 * P, D), np.float32)
    w2e[:PF] = W2
    w2e[PF] = bf2
    w2e = w2e.astype(BF16).reshape(33, P, D)

    in_maps = []
    for c in range(8):
        b, r = divmod(c, 4)
        hs = slice(r * 256, (r + 1) * 256)
        xT = np.ascontiguousarray(inputs[b].T)
        xte = np.zeros((9 * P, S), np.float32)
        xte[:D] = xT
        xte[D] = 1.0
        strips = np.concatenate(
            [np.arange(512 * j + P * r, 512 * j + P * r + P) for j in range(4)])
        mq = (padding_mask[b].astype(np.float32) * NEG).astype(BF16)
        mm = (1.0 - padding_mask[b].astype(np.float32)).T.astype(BF16)
        in_maps.append({
            "xt": xte.reshape(9, P, S),
            "xrows": inputs[b][strips] + bo[None, :],
            "wq": ext9(Wq[:, hs] / math.sqrt(DH), bq[hs] / math.sqrt(DH), 256),
            "wk": ext9(Wk[:, hs], bk[hs], 256),
            "wv": ext9(Wv[:, hs], bv[hs], 256),
            "wo": Wo[hs].astype(BF16).reshape(HL, DH, D),
            "w1": w1e,
            "w2": w2e,
            "maskq": np.ascontiguousarray(mq.reshape(16, P, S)),
            "maskm": np.ascontiguousarray(mm.reshape(16, P, S)),
            "identb": ident,
            "onesrow": onesr,
            "lng1": np.broadcast_to(ln1_g, (P, D)).copy(),
            "lnb1": np.broadcast_to(ln1_b, (P, D)).copy(),
            "lng2": np.broadcast_to(ln2_g, (P, D)).copy(),
            "lnb2": np.broadcast_to(ln2_b, (P, D)).copy(),
        })

    nc = _get_nc()
    res = run_bass_kernel_spmd(nc, in_maps, core_ids=list(range(8)),
                               **kw)
    out = np.empty((B, S, D), np.float32)
    attn = np.empty((B, H, S, S), np.float32)
    for c in range(8):
        b, r = divmod(c, 4)
        attn[b, r * HL:(r + 1) * HL] = res.results[c]["apr"]
        for j in range(4):
            out[b, 512 * j + P * r: 512 * j + P * r + P] = \
                res.results[c]["outp"][j * P:(j + 1) * P]
    kernel.last_results = res
    return out, attn
